# revision 1
# baseline (speedup 1.0000x reference)
"""Trainium2 Bass kernel for nn_Cholesky_from_z.

Reference computation (per batch sample b, n=128):
    s starts at 0 per row i; for column j: col = z[i,j]*sqrt(1-s) below diag,
    sqrt(1-s) on diag, 0 above; s += col^2.
Closed form: 1-s at (row i, col j) = prod_{k<j} (1 - z[i,k]^2), so
    L[i,j] = z[i,j] * sqrt(prod_{k<j}(1-z[i,k]^2))   (j < i)
    L[i,i] =          sqrt(prod_{k<i}(1-z[i,k]^2))
i.e. an exclusive cumulative product of (1-z^2) along each matrix row,
independent per row and per sample.

Device mapping: each sample's strictly-lower entries are packed row-major with
a 1.0 sentinel appended after each row (the "diagonal slot"), 8256 slots total.
One leading 1.0 column is prepended so every chunk can read one element back
for the shift.  On device, per [128 samples x chunk] tile:
    u = Square(z)                      (ACT)
    a = 1 - u, shifted one slot right  (DVE tensor_scalar)
        -> a = 0 exactly at each row-start slot (previous slot is the 1.0
           sentinel), which marks segment boundaries for free
    b = (a == 0) ? 1 : 0               (DVE, computed once; reused)
    d = scan: state = a*state + b      (DVE tensor_tensor_scan = segmented
                                        exclusive cumprod, carried across
                                        chunks via `initial`)
    q = Sqrt(d)                        (ACT)
    out = z * q                        (DVE)  [diag slot: 1 * q = q]
Batch dim (2048) is sharded 256 samples per core across 8 cores; each core
processes 2 partition-blocks of 128 samples.
"""

import sys

if "/opt/trn_rl_repo" not in sys.path:
    sys.path.insert(0, "/opt/trn_rl_repo")

import numpy as np

B = 2048
N = 128
NZ = N * (N - 1) // 2          # 8128 strictly-lower entries
PACKED = NZ + N                # 8256 slots incl. diagonal sentinels
NCORES = 8
B_CORE = B // NCORES           # 256
# ramp chunk schedule: small first/last chunks shorten pipeline fill/drain
CHUNKS = [1376, 2752, 2752, 1376]          # sums to PACKED (8256)
CHUNK_OFF = [0, 1376, 4128, 6880]
CMAX = max(CHUNKS)

# --- host-side index maps ---------------------------------------------------
# packed slot order: row i -> [z[i,0..i-1], diag_i]; row-start offset i(i+1)/2
_rows, _cols = np.tril_indices(N, -1)                  # row-major strict lower
_strict_slots = (_rows * (_rows + 1) // 2 + _cols).astype(np.int64)
_diag_slots = (np.arange(N) * (np.arange(N) + 1) // 2 + np.arange(N)).astype(np.int64)
# position of each packed slot in the dense [128,128] row-major output
_out_pos = np.empty(PACKED, np.int64)
_out_pos[_strict_slots] = _rows * N + _cols
_out_pos[_diag_slots] = np.arange(N) * N + np.arange(N)

_prog_cache = {}


def _build_program():
    import concourse.bacc as bacc
    import concourse.mybir as mybir
    from concourse.tile import TileContext

    f32 = mybir.dt.float32
    Alu = mybir.AluOpType
    Act = mybir.ActivationFunctionType

    nc = bacc.Bacc("TRN2", target_bir_lowering=False, debug=False,
                   num_devices=NCORES)
    zp = nc.dram_tensor("zp", [B_CORE, PACKED + 1], f32,
                        kind="ExternalInput").ap()
    lp = nc.dram_tensor("lp", [B_CORE, PACKED], f32,
                        kind="ExternalOutput").ap()

    NBLK = B_CORE // 128
    with TileContext(nc) as tc:
        with (
            tc.tile_pool(name="io", bufs=3) as io_pool,
            tc.tile_pool(name="up", bufs=2) as u_pool,
            tc.tile_pool(name="wq", bufs=2) as wq_pool,
            tc.tile_pool(name="dp", bufs=2) as dpool,
            tc.tile_pool(name="lt", bufs=3) as lt_pool,
            tc.tile_pool(name="bpool", bufs=1) as bpool,
        ):
            # DVE: scan + final multiply + b only; ACT: Square, 1-u, Sqrt.
            # GPSIMD untouched: concurrent GPSIMD inflates every engine ~20%.
            btiles = {}
            for blk in range(NBLK):
                r0 = blk * 128
                dprev = None
                for ch, (C, c0) in enumerate(zip(CHUNKS, CHUNK_OFF)):
                    zt = io_pool.tile([128, CMAX + 1], f32, tag="zt")
                    nc.sync.dma_start(out=zt[:, 0:C + 1],
                                      in_=zp[r0:r0 + 128, c0:c0 + C + 1])

                    # b = (prev slot == 1.0): boundary iff previous slot is
                    # the 1.0 diagonal sentinel (data slots are |z|<0.9).
                    # depends only on zt -> off the a/scan critical path.
                    if blk == 0:
                        bt = bpool.tile([128, CMAX], f32, tag=f"b{ch}")
                        nc.vector.tensor_scalar(bt[:, 0:C], zt[:, 0:C], 1.0,
                                                None, Alu.is_equal)
                        btiles[ch] = bt
                    bt = btiles[ch]

                    u = u_pool.tile([128, CMAX + 1], f32, tag="u")
                    nc.scalar.activation(u[:, 0:C + 1], zt[:, 0:C + 1],
                                         Act.Square)

                    # a[t] = 1 - u[t]  (u[t] already the shifted square)
                    a = wq_pool.tile([128, CMAX], f32, tag="w")
                    nc.scalar.activation(a[:, 0:C], u[:, 0:C],
                                         Act.Copy, bias=1.0, scale=-1.0)

                    d = dpool.tile([128, CMAX], f32, tag="d")
                    init = 1.0 if ch == 0 else dprev[0][:, dprev[1] - 1:dprev[1]]
                    nc.vector.tensor_tensor_scan(d[:, 0:C], a[:, 0:C],
                                                 bt[:, 0:C], init,
                                                 Alu.mult, Alu.add)
                    dprev = (d, C)

                    # a dead after scan; q reuses its slots (same tag)
                    q = wq_pool.tile([128, CMAX], f32, tag="w")
                    nc.scalar.activation(q[:, 0:C], d[:, 0:C], Act.Sqrt)

                    lt = lt_pool.tile([128, CMAX], f32, tag="lt")
                    nc.vector.tensor_mul(lt[:, 0:C], zt[:, 1:C + 1], q[:, 0:C])
                    nc.sync.dma_start(out=lp[r0:r0 + 128, c0:c0 + C],
                                      in_=lt[:, 0:C])
    nc.compile()
    return nc


def _get_program():
    if "nc" not in _prog_cache:
        _prog_cache["nc"] = _build_program()
    return _prog_cache["nc"]


def _run(in_maps, **kw):
    from concourse.bass_utils import run_bass_kernel_spmd

    nc = _get_program()
    return run_bass_kernel_spmd(nc, in_maps, list(range(NCORES)), **kw)


def kernel(inputs: np.ndarray, _return_raw=False, **run_kw) -> np.ndarray:
    assert inputs.shape == (B, NZ), inputs.shape
    zvec = np.ascontiguousarray(inputs, dtype=np.float32)

    # pack: one leading 1.0 column (shift sentinel) + per-row
    # [z..., 1.0 sentinel]
    zp = np.ones((B, PACKED + 1), np.float32)
    zp[:, 1 + _strict_slots] = zvec

    in_maps = [
        {"zp": np.ascontiguousarray(zp[c * B_CORE:(c + 1) * B_CORE])}
        for c in range(NCORES)
    ]
    res = _run(in_maps, **run_kw)

    lp = np.empty((B, PACKED), np.float32)
    for c in range(NCORES):
        lp[c * B_CORE:(c + 1) * B_CORE] = res.results[c]["lp"]

    out = np.zeros((B, N * N), np.float32)
    out[:, _out_pos] = lp
    out = out.reshape(B, N, N)
    if _return_raw:
        return out, res
    return out



# revision 2
# speedup vs baseline: 1.6636x; 1.6636x over previous
"""Trainium2 Bass kernel for nn_Cholesky_from_z.

Closed form: L[i,j] = z[i,j] * sqrt(prod_{k<j}(1-z[i,k]^2)) (j<i),
L[i,i] = sqrt(prod_{k<i}(1-z[i,k]^2)) -- an exclusive cumulative product
of (1-z^2) along each matrix row, independent per row and sample.

This kernel computes it in LOG space so the serial per-element scan becomes
a matmul on the (otherwise idle) tensor engine:
    c = S^T @ ln(1-z^2),  L = z * exp(c)
where S is a constant block-diagonal strictly-upper mask with value 0.5
(folds the sqrt) that performs a segmented exclusive cumsum.

Layout is TRANSPOSED vs the batch: partition = packed matrix-row position,
free dim = sample. Matrix rows are bin-packed into 65 blocks of exactly 128
positions (pairs (i,126-i) have total length 128 incl. one diagonal
sentinel slot per row; block 63 = row 63 + 64 pad; block 64 = row 127), so
no row ever crosses a block boundary and the cumsum needs no carries.

Per block b (tiles [128 pos, 256 samples], all fp16 except PSUM):
    u = z*z                      DVE (fp16, 2-4x mode)
    w = Ln(-u + 1)               ACT
    c = S_b^T @ w                PE  (fp16 matmul, fp32 PSUM)
    g = Exp(c)                   ACT (reads PSUM)
    L = z*g                      DVE
Diagonal sentinel is 0.998 (kept <1 so Ln stays finite); host divides diag
outputs by it. I/O is fp16 throughout (tolerance 2e-2; measured ~4e-4).
Batch dim 2048 is sharded 256 samples per core across 8 cores.
"""

import sys

if "/opt/trn_rl_repo" not in sys.path:
    sys.path.insert(0, "/opt/trn_rl_repo")

import numpy as np

B = 2048
N = 128
NZ = N * (N - 1) // 2          # 8128
NBLK = 65
PACK = NBLK * 128              # 8320
NCORES = 8
SAMP = B // NCORES             # 256
SENT = np.float16(0.998)
# dma/square/ln groups over blocks (ramped: small first for pipeline fill)
GROUPS = [(0, 4), (4, 16), (16, 32), (32, 48), (48, NBLK)]
BATCH = 4                      # matmul/exp/mult/dma-out granularity

# --- host-side packing maps -------------------------------------------------
def _build_maps():
    row_of_block = [(i, 126 - i) for i in range(63)] + [(63,), (127,)]
    slot_row = np.full(PACK, -1, np.int64)
    slot_col = np.full(PACK, -1, np.int64)
    for b, rows in enumerate(row_of_block):
        pos = b * 128
        for r in rows:
            L = r + 1
            slot_row[pos:pos + L] = r
            slot_col[pos:pos + L - 1] = np.arange(r)
            slot_col[pos + L - 1] = r          # diag/sentinel slot
            pos += L
    return slot_row, slot_col

_slot_row, _slot_col = _build_maps()
_valid = _slot_row >= 0
_strict = _valid & (_slot_col < _slot_row)
_diag = _valid & (_slot_col == _slot_row)
# index into the [B, 8128] row-major tril input for each strict slot
_tri_idx = (_slot_row[_strict] * (_slot_row[_strict] - 1) // 2
            + _slot_col[_strict])

def _build_S():
    seg = np.where(_valid, _slot_row, -1)
    S = np.zeros((NBLK, 128, 128), np.float16)
    k = np.arange(128)
    for b in range(NBLK):
        s = seg[b * 128:(b + 1) * 128]
        same = (s[:, None] == s[None, :]) & (s[:, None] >= 0)
        S[b] = np.where(same & (k[:, None] < k[None, :]),
                        np.float16(0.5), np.float16(0))
    # lhsT layout [k partition, block, t]
    return np.ascontiguousarray(S.transpose(1, 0, 2))

_S_host = _build_S()

_prog_cache = {}


def _build_program():
    import concourse.bacc as bacc
    import concourse.mybir as mybir
    from concourse.tile import TileContext

    f16 = mybir.dt.float16
    f32 = mybir.dt.float32
    Act = mybir.ActivationFunctionType

    nc = bacc.Bacc("TRN2", target_bir_lowering=False, debug=False,
                   num_devices=NCORES)
    zp = nc.dram_tensor("zp", [128, NBLK, SAMP], f16, kind="ExternalInput").ap()
    sc = nc.dram_tensor("sc", [128, NBLK, 128], f16, kind="ExternalInput").ap()
    lp = nc.dram_tensor("lp", [128, NBLK, SAMP], f16, kind="ExternalOutput").ap()

    with TileContext(nc) as tc:
        with (
            tc.tile_pool(name="sb", bufs=1) as sb,
            tc.psum_pool(name="ps", bufs=4) as pp,
        ):
            zt = sb.tile([128, NBLK, SAMP], f16)
            ut = sb.tile([128, NBLK, SAMP], f16)
            wt = sb.tile([128, NBLK, SAMP], f16)
            gt = sb.tile([128, NBLK, SAMP], f16)
            lt = sb.tile([128, NBLK, SAMP], f16)
            st = sb.tile([128, NBLK, 128], f16)

            done = 0
            for g0, g1 in GROUPS:
                nc.sync.dma_start(out=st[:, g0:g1, :], in_=sc[:, g0:g1, :])
                nc.sync.dma_start(out=zt[:, g0:g1, :], in_=zp[:, g0:g1, :])
                nc.vector.tensor_mul(ut[:, g0:g1, :], zt[:, g0:g1, :],
                                     zt[:, g0:g1, :])
                nc.scalar.activation(wt[:, g0:g1, :], ut[:, g0:g1, :],
                                     Act.Ln, bias=1.0, scale=-1.0)
                while done < g1 and (g1 - done >= BATCH or g1 == NBLK):
                    b0, b1 = done, min(done + BATCH, g1)
                    nb = b1 - b0
                    pt = pp.tile([128, BATCH, SAMP], f32, tag="ps")
                    for j in range(nb):
                        nc.tensor.matmul(pt[:, j, :], st[:, b0 + j, :],
                                         wt[:, b0 + j, :])
                    nc.scalar.activation(gt[:, b0:b1, :], pt[:, 0:nb, :],
                                         Act.Exp)
                    nc.vector.tensor_mul(lt[:, b0:b1, :], zt[:, b0:b1, :],
                                         gt[:, b0:b1, :])
                    nc.sync.dma_start(out=lp[:, b0:b1, :], in_=lt[:, b0:b1, :])
                    done = b1
    nc.compile()
    return nc


def _get_program():
    if "nc" not in _prog_cache:
        _prog_cache["nc"] = _build_program()
    return _prog_cache["nc"]


def kernel(inputs: np.ndarray, _return_raw=False, **run_kw) -> np.ndarray:
    from concourse.bass_utils import run_bass_kernel_spmd

    assert inputs.shape == (B, NZ), inputs.shape
    zvec = inputs.astype(np.float16)

    # pack [B, PACK]: strict-lower entries + 0.998 sentinels + 0 pad
    zpk = np.zeros((B, PACK), np.float16)
    zpk[:, _strict] = zvec[:, _tri_idx]
    zpk[:, _diag] = SENT

    in_maps = []
    for c in range(NCORES):
        zc = zpk[c * SAMP:(c + 1) * SAMP]            # [256, PACK]
        zc = zc.T.reshape(NBLK, 128, SAMP).transpose(1, 0, 2)
        in_maps.append({"zp": np.ascontiguousarray(zc), "sc": _S_host})

    nc = _get_program()
    res = run_bass_kernel_spmd(nc, in_maps, list(range(NCORES)), **run_kw)

    lpk = np.empty((B, PACK), np.float32)
    for c in range(NCORES):
        lc = res.results[c]["lp"]                    # [128, NBLK, SAMP]
        lpk[c * SAMP:(c + 1) * SAMP] = (
            lc.transpose(1, 0, 2).reshape(PACK, SAMP).T)

    out = np.zeros((B, N, N), np.float32)
    out[:, _slot_row[_strict], _slot_col[_strict]] = lpk[:, _strict]
    out[:, _slot_row[_diag], _slot_col[_diag]] = (
        lpk[:, _diag] / np.float32(SENT))
    if _return_raw:
        return out, res
    return out


# revision 3
# speedup vs baseline: 1.8052x; 1.0851x over previous
"""Trainium2 Bass kernel for nn_Cholesky_from_z.

Closed form: L[i,j] = z[i,j] * sqrt(prod_{k<j}(1-z[i,k]^2)) (j<i),
L[i,i] = sqrt(prod_{k<i}(1-z[i,k]^2)) -- an exclusive cumulative product
of (1-z^2) along each matrix row, independent per row and sample.

This kernel computes it in LOG space so the serial per-element scan becomes
a matmul on the (otherwise idle) tensor engine:
    c = S^T @ ln(1-z^2),  L = z * exp(c)
where S is a constant block-diagonal strictly-upper mask with value 0.5
(folds the sqrt) that performs a segmented exclusive cumsum.

Layout is TRANSPOSED vs the batch: partition = packed matrix-row position,
free dim = sample. Matrix rows are bin-packed into 65 blocks of exactly 128
positions (pairs (i,126-i) have total length 128 incl. one diagonal
sentinel slot per row; block 63 = row 63 + 64 pad; block 64 = row 127), so
no row ever crosses a block boundary and the cumsum needs no carries.

Per block b (tiles [128 pos, 256 samples], all fp16 except PSUM):
    u = z*z                      DVE (fp16, 2-4x mode)
    w = Ln(-u + 1)               ACT
    c = S_b^T @ w                PE  (fp16 matmul, fp32 PSUM)
    g = Exp(c)                   ACT (reads PSUM)
    L = z*g                      DVE
Diagonal sentinel is 0.998 (kept <1 so Ln stays finite); host divides diag
outputs by it. I/O is fp16 throughout (tolerance 2e-2; measured ~4e-4).
Batch dim 2048 is sharded 256 samples per core across 8 cores.
"""

import sys

if "/opt/trn_rl_repo" not in sys.path:
    sys.path.insert(0, "/opt/trn_rl_repo")

import numpy as np

B = 2048
N = 128
NZ = N * (N - 1) // 2          # 8128
NBLK = 65
PACK = NBLK * 128              # 8320
NCORES = 8
SAMP = B // NCORES             # 256
SENT = np.float16(0.998)
# dma/square/ln groups over blocks (ramped: small first for pipeline fill)
GROUPS = [(0, 4), (4, 16), (16, 32), (32, 48), (48, NBLK)]
BATCH = 4                      # matmul/exp/mult/dma-out granularity

# --- host-side packing maps -------------------------------------------------
def _build_maps():
    row_of_block = [(i, 126 - i) for i in range(63)] + [(63,), (127,)]
    slot_row = np.full(PACK, -1, np.int64)
    slot_col = np.full(PACK, -1, np.int64)
    for b, rows in enumerate(row_of_block):
        pos = b * 128
        for r in rows:
            L = r + 1
            slot_row[pos:pos + L] = r
            slot_col[pos:pos + L - 1] = np.arange(r)
            slot_col[pos + L - 1] = r          # diag/sentinel slot
            pos += L
    return slot_row, slot_col

_slot_row, _slot_col = _build_maps()
_valid = _slot_row >= 0
_strict = _valid & (_slot_col < _slot_row)
_diag = _valid & (_slot_col == _slot_row)
# index into the [B, 8128] row-major tril input for each strict slot
_tri_idx = (_slot_row[_strict] * (_slot_row[_strict] - 1) // 2
            + _slot_col[_strict])

def _build_S():
    seg = np.where(_valid, _slot_row, -1)
    S = np.zeros((NBLK, 128, 128), np.float16)
    k = np.arange(128)
    for b in range(NBLK):
        s = seg[b * 128:(b + 1) * 128]
        same = (s[:, None] == s[None, :]) & (s[:, None] >= 0)
        S[b] = np.where(same & (k[:, None] < k[None, :]),
                        np.float16(0.5), np.float16(0))
    # lhsT layout [k partition, block, t]
    return np.ascontiguousarray(S.transpose(1, 0, 2))

_S_host = _build_S()

_prog_cache = {}


def _build_program():
    import concourse.bacc as bacc
    import concourse.mybir as mybir
    from concourse.tile import TileContext

    f16 = mybir.dt.float16
    f32 = mybir.dt.float32
    Act = mybir.ActivationFunctionType

    nc = bacc.Bacc("TRN2", target_bir_lowering=False, debug=False,
                   num_devices=NCORES)
    zp = nc.dram_tensor("zp", [128, NBLK, SAMP], f16, kind="ExternalInput").ap()
    sc = nc.dram_tensor("sc", [128, NBLK, 128], f16, kind="ExternalInput").ap()
    lp = nc.dram_tensor("lp", [128, NBLK, SAMP], f16, kind="ExternalOutput").ap()

    with TileContext(nc) as tc:
        with (
            tc.tile_pool(name="sb", bufs=1) as sb,
            tc.psum_pool(name="ps", bufs=4) as pp,
        ):
            # preload the one act table that serves BOTH Ln and Exp
            # (set 6 = natural_log_exp_and_others); without this the
            # greedy table pass flip-flops Ln<->Exp tables (8 x 1.28us).
            import bass_rust as _br
            _tl = _br.InstLoadActFuncSet(
                name=nc.get_next_instruction_name(), ins=[], outs=[],
                act_func_set_id=6)
            nc.scalar.add_instruction(_tl)

            zt = sb.tile([128, NBLK, SAMP], f16)
            ut = sb.tile([128, NBLK, SAMP], f16)
            wt = sb.tile([128, NBLK, SAMP], f16)
            gt = sb.tile([128, NBLK, SAMP], f16)
            lt = sb.tile([128, NBLK, SAMP], f16)
            st = sb.tile([128, NBLK, 128], f16)

            done = 0
            for g0, g1 in GROUPS:
                nc.sync.dma_start(out=st[:, g0:g1, :], in_=sc[:, g0:g1, :])
                nc.sync.dma_start(out=zt[:, g0:g1, :], in_=zp[:, g0:g1, :])
                nc.vector.tensor_mul(ut[:, g0:g1, :], zt[:, g0:g1, :],
                                     zt[:, g0:g1, :])
                nc.scalar.activation(wt[:, g0:g1, :], ut[:, g0:g1, :],
                                     Act.Ln, bias=1.0, scale=-1.0)
                while done < g1 and (g1 - done >= BATCH or g1 == NBLK):
                    b0, b1 = done, min(done + BATCH, g1)
                    nb = b1 - b0
                    pt = pp.tile([128, BATCH, SAMP], f32, tag="ps")
                    for j in range(nb):
                        nc.tensor.matmul(pt[:, j, :], st[:, b0 + j, :],
                                         wt[:, b0 + j, :])
                    nc.scalar.activation(gt[:, b0:b1, :], pt[:, 0:nb, :],
                                         Act.Exp)
                    nc.vector.tensor_mul(lt[:, b0:b1, :], zt[:, b0:b1, :],
                                         gt[:, b0:b1, :])
                    nc.sync.dma_start(out=lp[:, b0:b1, :], in_=lt[:, b0:b1, :])
                    done = b1
    nc.compile()
    return nc


def _get_program():
    if "nc" not in _prog_cache:
        _prog_cache["nc"] = _build_program()
    return _prog_cache["nc"]


def kernel(inputs: np.ndarray, _return_raw=False, **run_kw) -> np.ndarray:
    from concourse.bass_utils import run_bass_kernel_spmd

    assert inputs.shape == (B, NZ), inputs.shape
    zvec = inputs.astype(np.float16)

    # pack [B, PACK]: strict-lower entries + 0.998 sentinels + 0 pad
    zpk = np.zeros((B, PACK), np.float16)
    zpk[:, _strict] = zvec[:, _tri_idx]
    zpk[:, _diag] = SENT

    in_maps = []
    for c in range(NCORES):
        zc = zpk[c * SAMP:(c + 1) * SAMP]            # [256, PACK]
        zc = zc.T.reshape(NBLK, 128, SAMP).transpose(1, 0, 2)
        in_maps.append({"zp": np.ascontiguousarray(zc), "sc": _S_host})

    nc = _get_program()
    res = run_bass_kernel_spmd(nc, in_maps, list(range(NCORES)), **run_kw)

    lpk = np.empty((B, PACK), np.float32)
    for c in range(NCORES):
        lc = res.results[c]["lp"]                    # [128, NBLK, SAMP]
        lpk[c * SAMP:(c + 1) * SAMP] = (
            lc.transpose(1, 0, 2).reshape(PACK, SAMP).T)

    out = np.zeros((B, N, N), np.float32)
    out[:, _slot_row[_strict], _slot_col[_strict]] = lpk[:, _strict]
    out[:, _slot_row[_diag], _slot_col[_diag]] = (
        lpk[:, _diag] / np.float32(SENT))
    if _return_raw:
        return out, res
    return out
